# revision 11
# baseline (speedup 1.0000x reference)
"""Trainium2 Bass kernel for nn_AWGNIndexChannelWrapper.

Reference computation:
  rx_c = bitflip(idx_c, flip_u_c)  (9-bit symbols, per-bit XOR with (u < BER))
  rx_f = bitflip(idx_f, flip_u_f)
  out  = concat([codebook_f[rx_f].reshape(B, -1), codebook_c[rx_c].reshape(B, -1)], axis=1)

Key simplification: packing/unpacking 9-bit symbols with per-bit XOR is just
  rx = idx ^ flipmask,  flipmask = sum_k (u_k < BER) << k
and the clip is a no-op (9-bit values are already < 512).

Sharding: pure data parallel over the batch dim (64 batches -> 8 cores x 8).

v2 design: the output is written with kv_writeback instead of plain DMA.
A kv_writeback with out view [b, dhi=128, dho, n_ctx=128], ncn = n_ctx = 128,
ctx_idx = 0 writes, for batch b, src[dhi, dho, b, :] to the contiguous DRAM
run at (dhi*dho_cnt + dho)*128.  With dho_cnt = points-per-batch/128 this is
exactly the row-major [point, D] layout the reference produces, and the SWDGE
descriptor stream covers a 16-partition stripe per descriptor, so the whole
20.97MB per-core output costs ~3.6us of DMA instead of ~58us.

The gathered data must sit in SBUF as [dhi=partition, b, dho, j] (the in_ap
passed to kv_writeback is the [dhi, dho, b, j] transpose of that physical
tile; kv_writeback only uses the AP's iteration order).  Point q of batch b
lives at partition q // dho_cnt, slot q % dho_cnt -- which is precisely the
layout dma_gather produces if gather position g = c*128 + p maps to point
q = p*dho_cnt + c.  That fixes the wrapped index layout to
  W[r, col = S*8 + j2] = rx[b, q]   (S = global slot, p = 16*j2 + r)
with per-region digits (fine: dho_cnt=32): q = 512*j2 + 32*r + c.

rx is therefore computed with partition P = 16*b + r and free dims (j2, c):
the idx/flip_u loads for that layout keep >=256B descriptor runs, the
(j2,c)->(c,j2) reorder is a free on-chip AP permute fused into the i32->i16
copy, and one DRAM round trip (write [r, b, c, j2] contiguous-per-partition,
read back with a x8 zero-stride partition-group broadcast) builds W.
"""

import os

import numpy as np

import concourse.bacc as bacc
import concourse.mybir as mybir
import concourse.tile as tile
from concourse.bass_utils import run_bass_kernel_spmd

# Problem constants (hardcoded per harness contract).
BER = 0.02
BITS = 9
KC = KF = 512
B, HC, WC, HF, WF, D = 64, 32, 32, 64, 64, 128

N_CORES = 8
B_LOC = B // N_CORES          # 8 batches per core
NF = B_LOC * HF * WF          # 32768 fine points per core
NC_ = B_LOC * HC * WC         # 8192 coarse points per core
P = 128
QF = HF * WF                  # 4096 fine points per batch
QC = HC * WC                  # 1024 coarse points per batch
DHOF = QF // P                # 32 fine slots per batch (d_head_outer)
DHOC = QC // P                # 8 coarse slots per batch
FINE_ROW = QF * D             # 524288 f32 per output row (fine region)
COARSE_ROW = QC * D           # 131072 f32 per output row (coarse region)
OUT_ROW = FINE_ROW + COARSE_ROW

f32 = mybir.dt.float32
i32 = mybir.dt.int32
i16 = mybir.dt.int16

J2 = 8                        # partition-group digit (p = 16*j2 + r)
R16 = 16                      # wrapped-row digit


def _region(nc, pool, dram_pool, out, kv_tile, ctx_t, *, idx_dram, flip_dram,
            cb, dho, row0, tag, n_queues, call_counter):
    """Full pipeline for one region (coarse or fine).

    Layouts (per batch b, point q = 512*j2 + 32*r + c for fine / analogous
    for coarse with strides (128, 8, 1)):
      compute tiles:  partition 16*b + r, free (j2, c)
      scratch SA:     [r, b, (c, j2)]  (contiguous 2*dho bytes * ... runs)
      W (wrapped):    [16*grp + r, 8*dho*b + 8*c + j2], grp-replicated
      gather dst:     kv_tile[:, b, c, :]
    """
    C = dho                    # c digit size
    FQ = J2 * C                # free size of compute tiles (per partition)
    WCOLS = B_LOC * J2 * C     # W columns
    f16 = mybir.dt.float16

    idx_t = pool.tile([P, FQ], i32, tag=f"idx{tag}")
    idx_flat = idx_dram.ap().rearrange("b h w -> b (h w)")
    u_flat = flip_dram.ap().rearrange("b h w k -> b (h w) k")
    u_t = pool.tile([P, FQ * BITS], f32, tag=f"u{tag}")
    for b in range(B_LOC):
        # idx on SP, u on ACT: HWDGE is shared but the two SEQs issue in
        # parallel, halving the serialized setup phase.
        nc.sync.dma_start(
            idx_t[16 * b : 16 * b + 16, :],
            idx_flat[b].rearrange("(j2 r c) -> r j2 c", j2=J2, r=R16),
        )
        nc.scalar.dma_start(
            u_t[16 * b : 16 * b + 16, :],
            u_flat[b].rearrange("(j2 r c) k -> r j2 (c k)", j2=J2, r=R16),
        )

    # flipmask: sc[p, f, k] = (u < BER) * 2^k (f16: values <= 256 exact),
    # reduce-add over k (sums <= 511, exact in f16)
    sc_t = pool.tile([P, FQ * BITS], f16, tag=f"sc{tag}")
    u_v = u_t[:].rearrange("p (f k) -> p f k", k=BITS)
    sc_v = sc_t[:].rearrange("p (f k) -> p f k", k=BITS)
    for k in range(BITS):
        nc.vector.tensor_scalar(
            out=sc_v[:, :, k],
            in0=u_v[:, :, k],
            scalar1=BER,
            scalar2=float(1 << k),
            op0=mybir.AluOpType.is_lt,
            op1=mybir.AluOpType.mult,
        )
    fm_t = pool.tile([P, FQ], f16, tag=f"fm{tag}")
    with nc.allow_low_precision(reason="bit sums <= 511 are exact in f16"):
        nc.vector.tensor_reduce(
            out=fm_t[:],
            in_=sc_t[:].rearrange("p (f k) -> p f k", k=BITS),
            axis=mybir.AxisListType.X,
            op=mybir.AluOpType.add,
        )
    fm_i = pool.tile([P, FQ], i32, tag=f"fmi{tag}")
    nc.vector.tensor_copy(out=fm_i[:], in_=fm_t[:])
    rx_t = pool.tile([P, FQ], i32, tag=f"rx{tag}")
    nc.vector.tensor_tensor(
        out=rx_t[:], in0=idx_t[:], in1=fm_i[:], op=mybir.AluOpType.bitwise_xor
    )
    # (j2, c) -> (c, j2) free permute fused into the i32 -> i16 copy so the
    # scratch write below is contiguous on both sides.
    rx16 = pool.tile([P, FQ], i16, tag=f"rx16{tag}")
    nc.vector.tensor_copy(
        out=rx16[:].rearrange("p (c j2) -> p c j2", j2=J2),
        in_=rx_t[:].rearrange("p (j2 c) -> p c j2", j2=J2),
    )

    # DRAM round trip to the wrapped, group-replicated index tile W.
    # SA[r, b, (c, j2)]: partition 16b+r writes one contiguous 2*FQ-byte run.
    sa = dram_pool.tile([R16, B_LOC, FQ], i16, tag=f"sa{tag}")
    nc.scalar.dma_start(sa[:].transpose([1, 0, 2]), rx16[:])
    W = pool.tile([P, WCOLS], i16, tag=f"W{tag}")
    nc.scalar.dma_start(
        W[:],
        sa[:]
        .rearrange("r b f -> r (b f)")
        .unsqueeze(0)
        .broadcast_to([J2, R16, WCOLS]),
    )

    # One gather call per batch: position g = c*128 + p lands at
    # kv_tile[p, b, c, :], reading W[g%16, b*8*dho + g//16].
    for b in range(B_LOC):
        n_idx = C * P
        nc.gpsimd.dma_gather(
            kv_tile[:, b, :, :],
            cb.ap(),
            W[:, b * J2 * C : (b + 1) * J2 * C],
            n_idx,
            n_idx,
            D,
            queue_num=call_counter[0] % n_queues,
            single_packet=False,
        )
        call_counter[0] += 1

    # kv_writeback: out[b, dhi, dho, 0:128] = kv_tile[dhi, b, dho, :], i.e.
    # the contiguous run at row b, offset row0 + (dhi*dho + c)*128.  One call
    # per batch: the interp materializes non-contiguous dst APs as packed
    # copies, so multi-batch calls (batch_stride != packed stride) misplace
    # batches > 0; a single-batch region is contiguous and always safe.
    out_v = (
        out.ap()[:, row0 : row0 + dho * P * D]
        .rearrange("b (dhi dho j) -> b dhi dho j", dhi=P, j=D)
    )
    for b in range(B_LOC):
        bs = slice(b, b + 1)
        nc.gpsimd.kv_writeback(
            out_v[bs],
            kv_tile[:, bs, :, :].transpose([0, 2, 1, 3]),
            ctx_t[:, bs],
            queue_num=call_counter[0] % n_queues,
        )
        call_counter[0] += 1


def build_nc():
    n_queues = int(os.environ.get("K_NQ", "4"))
    nc = bacc.Bacc(
        "TRN2", target_bir_lowering=False, debug=False, num_swdge_queues=n_queues,
        dynamic_dma_scratch_size=int(os.environ.get("K_RING", "65536")),
    )

    idx_c = nc.dram_tensor("idx_c", [B_LOC, HC, WC], i32, kind="ExternalInput")
    idx_f = nc.dram_tensor("idx_f", [B_LOC, HF, WF], i32, kind="ExternalInput")
    cb_c = nc.dram_tensor("codebook_c", [KC, D], f32, kind="ExternalInput")
    cb_f = nc.dram_tensor("codebook_f", [KF, D], f32, kind="ExternalInput")
    fu_c = nc.dram_tensor("flip_u_c", [B_LOC, HC, WC, BITS], f32, kind="ExternalInput")
    fu_f = nc.dram_tensor("flip_u_f", [B_LOC, HF, WF, BITS], f32, kind="ExternalInput")
    out = nc.dram_tensor("out", [B_LOC, OUT_ROW], f32, kind="ExternalOutput")

    with tile.TileContext(nc) as tc:
        with (
            tc.tile_pool(name="io", bufs=1) as pool,
            tc.tile_pool(name="dram", bufs=1, space="DRAM") as dram_pool,
        ):
            call_counter = [0]
            for _rep in range(int(os.environ.get("K_REPS", "1"))):
                ctx_t = pool.tile([P, B_LOC], i32, tag="ctx")
                nc.vector.memset(ctx_t[:], 0)
                kvf_t = pool.tile([P, B_LOC, DHOF, D], f32, tag="kvf")
                kvf = kvf_t[:]
                # The coarse region reuses the first DHOC slots of the fine
                # kv buffer (coarse writeback completes before fine gathers
                # overwrite it; subtile deps order the two).
                kvc = kvf[:, :, 0:DHOC, :]
                # Coarse first: its gathers feed the DMA engines while the
                # fine region's loads/bitflip prologue runs.
                _region(
                    nc, pool, dram_pool, out, kvc, ctx_t,
                    idx_dram=idx_c, flip_dram=fu_c, cb=cb_c, dho=DHOC,
                    row0=FINE_ROW, tag="c", n_queues=n_queues,
                    call_counter=call_counter,
                )
                _region(
                    nc, pool, dram_pool, out, kvf, ctx_t,
                    idx_dram=idx_f, flip_dram=fu_f, cb=cb_f, dho=DHOF,
                    row0=0, tag="f", n_queues=n_queues,
                    call_counter=call_counter,
                )

    nc.compile()
    return nc


_NC_CACHE = None


def _get_nc():
    global _NC_CACHE
    if _NC_CACHE is None:
        _NC_CACHE = build_nc()
    return _NC_CACHE


def _in_maps(idx_c, idx_f, codebook_c, codebook_f, flip_u_c, flip_u_f):
    maps = []
    for c in range(N_CORES):
        b0, b1 = c * B_LOC, (c + 1) * B_LOC
        maps.append(
            {
                "idx_c": np.ascontiguousarray(idx_c[b0:b1]),
                "idx_f": np.ascontiguousarray(idx_f[b0:b1]),
                "codebook_c": np.ascontiguousarray(codebook_c),
                "codebook_f": np.ascontiguousarray(codebook_f),
                "flip_u_c": np.ascontiguousarray(flip_u_c[b0:b1]),
                "flip_u_f": np.ascontiguousarray(flip_u_f[b0:b1]),
            }
        )
    return maps


class _AxonRunner:
    """Cached sharded PJRT executable for the axon path.

    run_bass_kernel_spmd rebuilds its jit closure (and retraces) on every
    call; caching the executable makes repeat kernel() calls cheap. Uses the
    same bass2jax machinery run_bass_kernel_spmd itself uses under axon.
    """

    def __init__(self, nc):
        import jax
        from jax.sharding import Mesh, NamedSharding, PartitionSpec
        from jax.experimental.shard_map import shard_map
        import concourse.bass2jax as b2j

        b2j.install_neuronx_cc_hook()
        self._jax = jax
        pname = nc.partition_id_tensor.name if nc.partition_id_tensor else None
        in_names, out_names, out_avals, zeros = [], [], [], []
        for alloc in nc.m.functions[0].allocations:
            if not isinstance(alloc, mybir.MemoryLocationSet):
                continue
            name = alloc.memorylocations[0].name
            if alloc.kind == "ExternalInput":
                if name != pname:
                    in_names.append(name)
            elif alloc.kind == "ExternalOutput":
                out_names.append(name)
                shape = tuple(alloc.tensor_shape)
                dtype = mybir.dt.np(alloc.dtype)
                out_avals.append(jax.core.ShapedArray(shape, dtype))
                zeros.append(np.zeros((N_CORES * shape[0], *shape[1:]), dtype))
        self.in_names = in_names
        all_in = in_names + out_names + ([pname] if pname else [])

        def _body(*args):
            ops = list(args)
            if pname is not None:
                ops.append(b2j.partition_id_tensor())
            return tuple(
                b2j._bass_exec_p.bind(
                    *ops,
                    out_avals=tuple(out_avals),
                    in_names=tuple(all_in),
                    out_names=tuple(out_names),
                    lowering_input_output_aliases=(),
                    sim_require_finite=True,
                    sim_require_nnan=True,
                    nc=nc,
                )
            )

        devices = jax.devices()[:N_CORES]
        mesh = Mesh(np.asarray(devices), ("core",))
        n = len(in_names) + len(out_names)
        self.sharded = jax.jit(
            shard_map(
                _body,
                mesh=mesh,
                in_specs=(PartitionSpec("core"),) * n,
                out_specs=(PartitionSpec("core"),) * len(out_names),
                check_rep=False,
            ),
            keep_unused=True,
        )
        self.sh = NamedSharding(mesh, PartitionSpec("core"))
        self.dev_zeros = [jax.device_put(z, self.sh) for z in zeros]

    def run(self, full):
        jax = self._jax
        dev_in = [jax.device_put(full[n], self.sh) for n in self.in_names]
        outs = self.sharded(*dev_in, *self.dev_zeros)
        return np.asarray(outs[0]).reshape(B, OUT_ROW)


_RUNNER = None


def kernel(idx_c, idx_f, codebook_c, codebook_f, flip_u_c, flip_u_f):
    from concourse._compat import axon_active

    if axon_active():
        global _RUNNER
        if _RUNNER is None:
            _RUNNER = _AxonRunner(_get_nc())
        full = {
            "idx_c": np.ascontiguousarray(idx_c),
            "idx_f": np.ascontiguousarray(idx_f),
            "codebook_c": np.tile(np.ascontiguousarray(codebook_c), (N_CORES, 1)),
            "codebook_f": np.tile(np.ascontiguousarray(codebook_f), (N_CORES, 1)),
            "flip_u_c": np.ascontiguousarray(flip_u_c),
            "flip_u_f": np.ascontiguousarray(flip_u_f),
        }
        return _RUNNER.run(full)

    nc = _get_nc()
    maps = _in_maps(idx_c, idx_f, codebook_c, codebook_f, flip_u_c, flip_u_f)
    res = run_bass_kernel_spmd(nc, maps, core_ids=list(range(N_CORES)))
    return np.concatenate([r["out"] for r in res.results], axis=0)
